# revision 17
# baseline (speedup 1.0000x reference)
"""Bass/Trainium2 kernel for nn_MultiHeadAttention (B=8, S=1024, H=12, D=64, model=768).

Sharding: pure data parallel over batch — core b computes batch element b.
Returns (out, weights) matching the reference.

Matmuls run in fp32r (full PE rate, ~1.6e-4 matmul relerr vs fp32's 4x slowdown).
Softmax (exp, row-sum, normalization) is exact fp32.
"""
import numpy as np
from contextlib import ExitStack

import concourse.bass as bass
import concourse.tile as tile
from concourse import bacc, mybir
from concourse.bass_utils import run_bass_kernel_spmd

F32 = mybir.dt.float32
F32R = mybir.dt.float32r

# Problem constants (hardcoded per contract)
B = 8
S = 1024
MD = 768          # model dim
H = 12            # heads
DK = 64           # head dim
KC = MD // 128    # 6 contraction chunks of 128
ST = S // 128     # 8 s1 tiles
NPAIR = H // 2    # 6 head pairs
SCALE = DK ** -0.5

_BUILD_CACHE = {}


def build_nc():
    """Build the SPMD per-core kernel. Same program on all 8 cores."""
    nc = bacc.Bacc("TRN2", target_bir_lowering=False, debug=False, num_devices=B)

    # ---- DRAM I/O ----  (matmul operands declared fp32r; same bytes as fp32)
    xT_d = nc.dram_tensor("xT", [MD, S], F32R, kind="ExternalInput")
    w_qk_d = nc.dram_tensor("w_qk", [H, 128, KC, 128], F32R, kind="ExternalInput")
    b_qk_d = nc.dram_tensor("b_qk", [128, H], F32, kind="ExternalInput")
    w_v_d = nc.dram_tensor("w_v", [KC, 128, MD], F32R, kind="ExternalInput")
    b_v_d = nc.dram_tensor("b_v", [128, MD], F32, kind="ExternalInput")
    w_out_d = nc.dram_tensor("w_out", [KC, 128, MD], F32R, kind="ExternalInput")
    b_out_d = nc.dram_tensor("b_out", [128, MD], F32, kind="ExternalInput")
    ident_d = nc.dram_tensor("ident", [128, 128], F32, kind="ExternalInput")

    out_d = nc.dram_tensor("out", [S, MD], F32, kind="ExternalOutput")
    wts_d = nc.dram_tensor("weights", [H, S, S], F32, kind="ExternalOutput")

    with tile.TileContext(nc) as tc:
        with ExitStack() as ctx:
            consts = ctx.enter_context(tc.tile_pool(name="consts", bufs=1))
            qkp = ctx.enter_context(tc.tile_pool(name="qkp", bufs=2))
            slabp = ctx.enter_context(tc.tile_pool(name="slabp", bufs=2))
            vp = ctx.enter_context(tc.tile_pool(name="vp", bufs=1))
            wtp = ctx.enter_context(tc.tile_pool(name="wtp", bufs=1))
            work = ctx.enter_context(tc.tile_pool(name="work", bufs=2))
            scal = ctx.enter_context(tc.tile_pool(name="scal", bufs=4))
            atp = ctx.enter_context(tc.tile_pool(name="atp", bufs=1))

            ps_big = ctx.enter_context(tc.tile_pool(name="ps_big", bufs=2, space="PSUM"))
            ps_t = ctx.enter_context(tc.tile_pool(name="ps_t", bufs=2, space="PSUM"))

            # ---- constant loads ----
            xT = [consts.tile([128, S], F32R, tag=f"xT{k}", name=f"xT{k}") for k in range(KC)]
            for k in range(KC):
                nc.sync.dma_start(out=xT[k], in_=xT_d[128 * k:128 * (k + 1), :])
            w_v = [consts.tile([128, MD], F32R, tag=f"wv{k}", name=f"wv{k}") for k in range(KC)]
            for k in range(KC):
                nc.sync.dma_start(out=w_v[k], in_=w_v_d[k])
            w_out = [consts.tile([128, MD], F32R, tag=f"wo{k}", name=f"wo{k}") for k in range(KC)]
            for k in range(KC):
                nc.sync.dma_start(out=w_out[k], in_=w_out_d[k])
            b_qk = consts.tile([128, H], F32, tag="bqk")
            nc.sync.dma_start(out=b_qk, in_=b_qk_d[:, :])
            b_v = consts.tile([128, MD], F32, tag="bv")
            nc.sync.dma_start(out=b_v, in_=b_v_d[:, :])
            b_out = consts.tile([128, MD], F32, tag="bout")
            nc.sync.dma_start(out=b_out, in_=b_out_d[:, :])
            ident = consts.tile([128, 128], F32, tag="id")
            nc.sync.dma_start(out=ident, in_=ident_d[:, :])

            NSL = [(0, 512), (512, MD - 512)]  # N slices for MD-wide matmuls

            # ---- V projection: v[s2, (h d)] = xT.T @ W_v + b_v ----
            v_t = [vp.tile([128, MD], F32R, tag=f"v{mt}", name=f"v{mt}") for mt in range(ST)]
            for mt in range(ST):
                ps = ps_t.tile([128, 1024], F32, tag="pt", name=f"vps{mt}")
                for k in range(KC):
                    for (n0, nn) in NSL:
                        nc.tensor.matmul(
                            ps[:, n0:n0 + nn],
                            xT[k][:, 128 * mt:128 * (mt + 1)],
                            w_v[k][:, n0:n0 + nn],
                            start=(k == 0), stop=(k == KC - 1),
                        )
                nc.vector.tensor_add(v_t[mt], ps[:, :MD], b_v)

            # wT chunk layout: chunk j occupies cols [1024j, 1024(j+1))
            wT = wtp.tile([128, ST * 1024], F32R, tag="wT")
            wT_v = wT.rearrange("p (j q) -> p j q", j=ST)

            # ---- per head-pair ----
            at_all = []
            for c in range(NPAIR):
                slabq = slabp.tile([128, MD], F32R, tag="slabq")
                nc.sync.dma_start(out=slabq, in_=w_qk_d[c].rearrange("p k c -> p (k c)"))
                slabk = slabp.tile([128, MD], F32R, tag="slabk")
                nc.sync.dma_start(out=slabk, in_=w_qk_d[NPAIR + c].rearrange("p k c -> p (k c)"))

                qt = qkp.tile([128, S], F32R, tag="qt")
                kt = qkp.tile([128, S], F32R, tag="kt")
                for dst, slab, bi in ((qt, slabq, c), (kt, slabk, NPAIR + c)):
                    ps = ps_big.tile([128, 1024], F32, tag="big", name="qkps")
                    for k in range(KC):
                        for nt in range(2):
                            nc.tensor.matmul(
                                ps[:, 512 * nt:512 * (nt + 1)],
                                slab[:, 128 * k:128 * (k + 1)],
                                xT[k][:, 512 * nt:512 * (nt + 1)],
                                start=(k == 0), stop=(k == KC - 1),
                            )
                    nc.vector.tensor_scalar_add(dst, ps, b_qk[:, bi:bi + 1])

                at = atp.tile([128, S], F32R, tag=f"at{c}", name=f"at{c}")
                for r in range(2):
                    h = 2 * c + r
                    for s1t in range(ST):
                        sc = ps_big.tile([128, 1024], F32, tag="big", name="scps")
                        for nt in range(2):
                            nc.tensor.matmul(
                                sc[:, 512 * nt:512 * (nt + 1)],
                                qt[64 * r:64 * (r + 1), 128 * s1t:128 * (s1t + 1)],
                                kt[64 * r:64 * (r + 1), 512 * nt:512 * (nt + 1)],
                                start=True, stop=True,
                            )
                        e = work.tile([128, 1024], F32, tag="e")
                        ssum = scal.tile([128, 1], F32, tag="ss")
                        nc.scalar.activation(
                            e, sc, mybir.ActivationFunctionType.Exp,
                            bias=0.0, scale=SCALE, accum_out=ssum,
                        )
                        rcp = scal.tile([128, 1], F32, tag="rc")
                        nc.vector.reciprocal(rcp, ssum)
                        wtile = work.tile([128, 1024], F32, tag="w")
                        nc.vector.tensor_scalar_mul(wtile, e, rcp)
                        nc.sync.dma_start(
                            out=wts_d[h, 128 * s1t:128 * (s1t + 1), :], in_=wtile
                        )
                        tp = ps_t.tile([128, 1024], F32, tag="pt", name="tps")
                        for j in range(ST):
                            nc.tensor.transpose(
                                tp[:, 128 * j:128 * (j + 1)],
                                wtile[:, 128 * j:128 * (j + 1)],
                                ident,
                            )
                        tp_v = tp.rearrange("p (j q) -> p j q", j=ST)
                        JS = 6  # DVE takes chunks 0..5, ACT takes 6..7
                        nc.vector.tensor_copy(
                            wT_v[:, :JS, 128 * s1t:128 * (s1t + 1)],
                            tp_v[:, :JS, :],
                        )
                        nc.scalar.copy(
                            wT_v[:, JS:, 128 * s1t:128 * (s1t + 1)],
                            tp_v[:, JS:, :],
                        )
                    # attention: attnT[d, s1] for head h (fp32r needs dst part 0)
                    pa = ps_t.tile([64, 1024], F32, tag="pt", name=f"pa{h}")
                    for nt in range(2):
                        for j in range(ST):
                            nc.tensor.matmul(
                                pa[:, 512 * nt:512 * (nt + 1)],
                                v_t[j][:, 64 * h:64 * (h + 1)],
                                wT_v[:, j, 512 * nt:512 * (nt + 1)],
                                start=(j == 0), stop=(j == ST - 1),
                            )
                    nc.scalar.copy(at[64 * r:64 * (r + 1), :], pa)
                at_all.append(at)

            # ---- output projection: out[s1, m] = attnT.T @ W_out + b_out ----
            for s1t in range(ST):
                ps = ps_t.tile([128, 1024], F32, tag="pt", name=f"ops{s1t}")
                for cc in range(KC):
                    for (n0, nn) in NSL:
                        nc.tensor.matmul(
                            ps[:, n0:n0 + nn],
                            at_all[cc][:, 128 * s1t:128 * (s1t + 1)],
                            w_out[cc][:, n0:n0 + nn],
                            start=(cc == 0), stop=(cc == KC - 1),
                        )
                osb = work.tile([128, MD], F32, tag="osb")
                nc.vector.tensor_add(osb, ps[:, :MD], b_out)
                nc.sync.dma_start(
                    out=out_d[128 * s1t:128 * (s1t + 1), :], in_=osb
                )

    nc.compile()
    return nc


def prep_inputs(x, W_qkv, b_qkv, W_out, b_out):
    """Host-side resharding/reordering. Returns per-core input maps."""
    x = np.asarray(x, dtype=np.float32)
    W_qkv = np.asarray(W_qkv, dtype=np.float32)
    b_qkv = np.asarray(b_qkv, dtype=np.float32)
    W_out = np.asarray(W_out, dtype=np.float32)
    b_out = np.asarray(b_out, dtype=np.float32)

    # original col index = h*(DK*3) + d*3 + comp
    cols = np.arange(H * DK * 3).reshape(H, DK, 3)
    idx_q = cols[:, :, 0].reshape(-1)     # (h, d) order
    idx_k = cols[:, :, 1].reshape(-1)
    idx_v = cols[:, :, 2].reshape(-1)

    W_QK = np.concatenate([W_qkv[:, idx_q], W_qkv[:, idx_k]], axis=1)  # [768, 1536]
    b_QK = np.concatenate([b_qkv[idx_q], b_qkv[idx_k]])                # [1536]
    W_V = W_qkv[:, idx_v]                                              # [768, 768]
    b_V = b_qkv[idx_v]

    # w_qk tiled: w_qk[m, p, k, cc] = W_QK[128k + p, 128m + cc]
    w_qk = np.ascontiguousarray(
        W_QK.reshape(KC, 128, H, 128).transpose(2, 1, 0, 3)
    )
    b_qk = np.ascontiguousarray(b_QK.reshape(H, 128).T)               # [128, 12]
    w_v = np.ascontiguousarray(W_V.reshape(KC, 128, MD))
    b_v = np.ascontiguousarray(np.broadcast_to(b_V, (128, MD)))
    w_out_t = np.ascontiguousarray(W_out.reshape(KC, 128, MD))
    b_out_b = np.ascontiguousarray(np.broadcast_to(b_out, (128, MD)))
    ident = np.eye(128, dtype=np.float32)

    shared = {
        "w_qk": w_qk, "b_qk": b_qk, "w_v": w_v, "b_v": b_v,
        "w_out": w_out_t, "b_out": b_out_b, "ident": ident,
    }
    in_maps = []
    for b in range(B):
        m = dict(shared)
        m["xT"] = np.ascontiguousarray(x[b].T)
        in_maps.append(m)
    return in_maps


def _get_nc():
    if "nc" not in _BUILD_CACHE:
        _BUILD_CACHE["nc"] = build_nc()
    return _BUILD_CACHE["nc"]


def run(inputs, trace=False, **kw):
    nc = _get_nc()
    in_maps = prep_inputs(**inputs)
    res = run_bass_kernel_spmd(nc, in_maps, core_ids=list(range(B)), trace=trace, **kw)
    out = np.stack([res.results[b]["out"] for b in range(B)])
    weights = np.stack([res.results[b]["weights"] for b in range(B)])
    return (out, weights), res


def kernel(x, W_qkv, b_qkv, W_out, b_out):
    (out, weights), _ = run(
        dict(x=x, W_qkv=W_qkv, b_qkv=b_qkv, W_out=W_out, b_out=b_out)
    )
    return out, weights


# revision 18
# speedup vs baseline: 1.0234x; 1.0234x over previous
"""Bass/Trainium2 kernel for nn_MultiHeadAttention (B=8, S=1024, H=12, D=64, model=768).

Sharding: pure data parallel over batch — core b computes batch element b.
Returns (out, weights) matching the reference.

Matmuls run in fp32r (full PE rate, ~1.6e-4 matmul relerr vs fp32's 4x slowdown).
Softmax (exp, row-sum, normalization) is exact fp32.
"""
import numpy as np
from contextlib import ExitStack

import concourse.bass as bass
import concourse.tile as tile
from concourse import bacc, mybir
from concourse.bass_utils import run_bass_kernel_spmd

F32 = mybir.dt.float32
F32R = mybir.dt.float32r

# Problem constants (hardcoded per contract)
B = 8
S = 1024
MD = 768          # model dim
H = 12            # heads
DK = 64           # head dim
KC = MD // 128    # 6 contraction chunks of 128
ST = S // 128     # 8 s1 tiles
NPAIR = H // 2    # 6 head pairs
SCALE = DK ** -0.5

_BUILD_CACHE = {}


def build_nc():
    """Build the SPMD per-core kernel. Same program on all 8 cores."""
    nc = bacc.Bacc("TRN2", target_bir_lowering=False, debug=False, num_devices=B)

    # ---- DRAM I/O ----  (matmul operands declared fp32r; same bytes as fp32)
    xT_d = nc.dram_tensor("xT", [MD, S], F32R, kind="ExternalInput")
    w_qk_d = nc.dram_tensor("w_qk", [H, 128, KC, 128], F32R, kind="ExternalInput")
    b_qk_d = nc.dram_tensor("b_qk", [128, H], F32, kind="ExternalInput")
    w_v_d = nc.dram_tensor("w_v", [KC, 128, MD], F32R, kind="ExternalInput")
    b_v_d = nc.dram_tensor("b_v", [128, MD], F32, kind="ExternalInput")
    w_out_d = nc.dram_tensor("w_out", [KC, 128, MD], F32R, kind="ExternalInput")
    b_out_d = nc.dram_tensor("b_out", [128, MD], F32, kind="ExternalInput")
    ident_d = nc.dram_tensor("ident", [128, 128], F32, kind="ExternalInput")

    out_d = nc.dram_tensor("out", [S, MD], F32, kind="ExternalOutput")
    wts_d = nc.dram_tensor("weights", [H, S, S], F32, kind="ExternalOutput")

    with tile.TileContext(nc) as tc:
        with ExitStack() as ctx:
            consts = ctx.enter_context(tc.tile_pool(name="consts", bufs=1))
            qkp = ctx.enter_context(tc.tile_pool(name="qkp", bufs=2))
            slabp = ctx.enter_context(tc.tile_pool(name="slabp", bufs=2))
            vp = ctx.enter_context(tc.tile_pool(name="vp", bufs=1))
            wtp = ctx.enter_context(tc.tile_pool(name="wtp", bufs=1))
            work = ctx.enter_context(tc.tile_pool(name="work", bufs=2))
            scal = ctx.enter_context(tc.tile_pool(name="scal", bufs=4))
            atp = ctx.enter_context(tc.tile_pool(name="atp", bufs=1))

            ps_big = ctx.enter_context(tc.tile_pool(name="ps_big", bufs=2, space="PSUM"))
            ps_t = ctx.enter_context(tc.tile_pool(name="ps_t", bufs=2, space="PSUM"))

            # ---- constant loads ----
            xT = [consts.tile([128, S], F32R, tag=f"xT{k}", name=f"xT{k}") for k in range(KC)]
            for k in range(KC):
                nc.sync.dma_start(out=xT[k], in_=xT_d[128 * k:128 * (k + 1), :])
            w_v = [consts.tile([128, MD], F32R, tag=f"wv{k}", name=f"wv{k}") for k in range(KC)]
            for k in range(KC):
                nc.sync.dma_start(out=w_v[k], in_=w_v_d[k])
            w_out = [consts.tile([128, MD], F32R, tag=f"wo{k}", name=f"wo{k}") for k in range(KC)]
            for k in range(KC):
                nc.sync.dma_start(out=w_out[k], in_=w_out_d[k])
            b_qk = consts.tile([128, H], F32, tag="bqk")
            nc.sync.dma_start(out=b_qk, in_=b_qk_d[:, :])
            b_v = consts.tile([128, MD], F32, tag="bv")
            nc.sync.dma_start(out=b_v, in_=b_v_d[:, :])
            b_out = consts.tile([128, MD], F32, tag="bout")
            nc.sync.dma_start(out=b_out, in_=b_out_d[:, :])
            ident = consts.tile([128, 128], F32, tag="id")
            nc.sync.dma_start(out=ident, in_=ident_d[:, :])

            NSL = [(0, 512), (512, MD - 512)]  # N slices for MD-wide matmuls

            # ---- V projection: v[s2, (h d)] = xT.T @ W_v + b_v ----
            v_t = [vp.tile([128, MD], F32R, tag=f"v{mt}", name=f"v{mt}") for mt in range(ST)]
            for mt in range(ST):
                ps = ps_t.tile([128, 1024], F32, tag="pt", name=f"vps{mt}")
                for k in range(KC):
                    for (n0, nn) in NSL:
                        nc.tensor.matmul(
                            ps[:, n0:n0 + nn],
                            xT[k][:, 128 * mt:128 * (mt + 1)],
                            w_v[k][:, n0:n0 + nn],
                            start=(k == 0), stop=(k == KC - 1),
                        )
                nc.vector.tensor_add(v_t[mt], ps[:, :MD], b_v)

            # wT chunk layout: chunk j occupies cols [1024j, 1024(j+1))
            wT = wtp.tile([128, ST * 1024], F32R, tag="wT")
            wT_v = wT.rearrange("p (j q) -> p j q", j=ST)

            # ---- per head-pair ----
            at_all = []
            for c in range(NPAIR):
                slabq = slabp.tile([128, MD], F32R, tag="slabq")
                nc.sync.dma_start(out=slabq, in_=w_qk_d[c].rearrange("p k c -> p (k c)"))
                slabk = slabp.tile([128, MD], F32R, tag="slabk")
                nc.sync.dma_start(out=slabk, in_=w_qk_d[NPAIR + c].rearrange("p k c -> p (k c)"))

                qt = qkp.tile([128, S], F32R, tag="qt")
                kt = qkp.tile([128, S], F32R, tag="kt")
                for dst, slab, bi in ((qt, slabq, c), (kt, slabk, NPAIR + c)):
                    ps = ps_big.tile([128, 1024], F32, tag="big", name="qkps")
                    for k in range(KC):
                        for nt in range(2):
                            nc.tensor.matmul(
                                ps[:, 512 * nt:512 * (nt + 1)],
                                slab[:, 128 * k:128 * (k + 1)],
                                xT[k][:, 512 * nt:512 * (nt + 1)],
                                start=(k == 0), stop=(k == KC - 1),
                            )
                    nc.scalar.add(dst, ps, b_qk[:, bi:bi + 1])

                at = atp.tile([128, S], F32R, tag=f"at{c}", name=f"at{c}")
                for r in range(2):
                    h = 2 * c + r
                    for s1t in range(ST):
                        sc = ps_big.tile([128, 1024], F32, tag="big", name="scps")
                        for nt in range(2):
                            nc.tensor.matmul(
                                sc[:, 512 * nt:512 * (nt + 1)],
                                qt[64 * r:64 * (r + 1), 128 * s1t:128 * (s1t + 1)],
                                kt[64 * r:64 * (r + 1), 512 * nt:512 * (nt + 1)],
                                start=True, stop=True,
                            )
                        e = work.tile([128, 1024], F32, tag="e")
                        ssum = scal.tile([128, 1], F32, tag="ss")
                        nc.scalar.activation(
                            e, sc, mybir.ActivationFunctionType.Exp,
                            bias=0.0, scale=SCALE, accum_out=ssum,
                        )
                        rcp = scal.tile([128, 1], F32, tag="rc")
                        nc.vector.reciprocal(rcp, ssum)
                        wtile = work.tile([128, 1024], F32, tag="w")
                        nc.vector.tensor_scalar_mul(wtile, e, rcp)
                        nc.sync.dma_start(
                            out=wts_d[h, 128 * s1t:128 * (s1t + 1), :], in_=wtile
                        )
                        tp = ps_t.tile([128, 1024], F32, tag="pt", name="tps")
                        for j in range(ST):
                            nc.tensor.transpose(
                                tp[:, 128 * j:128 * (j + 1)],
                                wtile[:, 128 * j:128 * (j + 1)],
                                ident,
                            )
                        tp_v = tp.rearrange("p (j q) -> p j q", j=ST)
                        JS = 5  # DVE takes chunks 0..4, ACT takes 5..7
                        nc.vector.tensor_copy(
                            wT_v[:, :JS, 128 * s1t:128 * (s1t + 1)],
                            tp_v[:, :JS, :],
                        )
                        nc.scalar.copy(
                            wT_v[:, JS:, 128 * s1t:128 * (s1t + 1)],
                            tp_v[:, JS:, :],
                        )
                    # attention: attnT[d, s1] for head h (fp32r needs dst part 0)
                    pa = ps_t.tile([64, 1024], F32, tag="pt", name=f"pa{h}")
                    for nt in range(2):
                        for j in range(ST):
                            nc.tensor.matmul(
                                pa[:, 512 * nt:512 * (nt + 1)],
                                v_t[j][:, 64 * h:64 * (h + 1)],
                                wT_v[:, j, 512 * nt:512 * (nt + 1)],
                                start=(j == 0), stop=(j == ST - 1),
                            )
                    nc.scalar.copy(at[64 * r:64 * (r + 1), :], pa)
                at_all.append(at)

            # ---- output projection: out[s1, m] = attnT.T @ W_out + b_out ----
            for s1t in range(ST):
                ps = ps_t.tile([128, 1024], F32, tag="pt", name=f"ops{s1t}")
                for cc in range(KC):
                    for (n0, nn) in NSL:
                        nc.tensor.matmul(
                            ps[:, n0:n0 + nn],
                            at_all[cc][:, 128 * s1t:128 * (s1t + 1)],
                            w_out[cc][:, n0:n0 + nn],
                            start=(cc == 0), stop=(cc == KC - 1),
                        )
                osb = work.tile([128, MD], F32, tag="osb")
                nc.vector.tensor_add(osb, ps[:, :MD], b_out)
                nc.sync.dma_start(
                    out=out_d[128 * s1t:128 * (s1t + 1), :], in_=osb
                )

    nc.compile()
    return nc


def prep_inputs(x, W_qkv, b_qkv, W_out, b_out):
    """Host-side resharding/reordering. Returns per-core input maps."""
    x = np.asarray(x, dtype=np.float32)
    W_qkv = np.asarray(W_qkv, dtype=np.float32)
    b_qkv = np.asarray(b_qkv, dtype=np.float32)
    W_out = np.asarray(W_out, dtype=np.float32)
    b_out = np.asarray(b_out, dtype=np.float32)

    # original col index = h*(DK*3) + d*3 + comp
    cols = np.arange(H * DK * 3).reshape(H, DK, 3)
    idx_q = cols[:, :, 0].reshape(-1)     # (h, d) order
    idx_k = cols[:, :, 1].reshape(-1)
    idx_v = cols[:, :, 2].reshape(-1)

    W_QK = np.concatenate([W_qkv[:, idx_q], W_qkv[:, idx_k]], axis=1)  # [768, 1536]
    b_QK = np.concatenate([b_qkv[idx_q], b_qkv[idx_k]])                # [1536]
    W_V = W_qkv[:, idx_v]                                              # [768, 768]
    b_V = b_qkv[idx_v]

    # w_qk tiled: w_qk[m, p, k, cc] = W_QK[128k + p, 128m + cc]
    w_qk = np.ascontiguousarray(
        W_QK.reshape(KC, 128, H, 128).transpose(2, 1, 0, 3)
    )
    b_qk = np.ascontiguousarray(b_QK.reshape(H, 128).T)               # [128, 12]
    w_v = np.ascontiguousarray(W_V.reshape(KC, 128, MD))
    b_v = np.ascontiguousarray(np.broadcast_to(b_V, (128, MD)))
    w_out_t = np.ascontiguousarray(W_out.reshape(KC, 128, MD))
    b_out_b = np.ascontiguousarray(np.broadcast_to(b_out, (128, MD)))
    ident = np.eye(128, dtype=np.float32)

    shared = {
        "w_qk": w_qk, "b_qk": b_qk, "w_v": w_v, "b_v": b_v,
        "w_out": w_out_t, "b_out": b_out_b, "ident": ident,
    }
    in_maps = []
    for b in range(B):
        m = dict(shared)
        m["xT"] = np.ascontiguousarray(x[b].T)
        in_maps.append(m)
    return in_maps


def _get_nc():
    if "nc" not in _BUILD_CACHE:
        _BUILD_CACHE["nc"] = build_nc()
    return _BUILD_CACHE["nc"]


def run(inputs, trace=False, **kw):
    nc = _get_nc()
    in_maps = prep_inputs(**inputs)
    res = run_bass_kernel_spmd(nc, in_maps, core_ids=list(range(B)), trace=trace, **kw)
    out = np.stack([res.results[b]["out"] for b in range(B)])
    weights = np.stack([res.results[b]["weights"] for b in range(B)])
    return (out, weights), res


def kernel(x, W_qkv, b_qkv, W_out, b_out):
    (out, weights), _ = run(
        dict(x=x, W_qkv=W_qkv, b_qkv=b_qkv, W_out=W_out, b_out=b_out)
    )
    return out, weights


# revision 19
# speedup vs baseline: 1.0520x; 1.0280x over previous
"""Bass/Trainium2 kernel for nn_MultiHeadAttention (B=8, S=1024, H=12, D=64, model=768).

Sharding: pure data parallel over batch — core b computes batch element b.
Returns (out, weights) matching the reference.

Matmuls run in fp32r (full PE rate, ~1.6e-4 matmul relerr vs fp32's 4x slowdown).
Softmax (exp, row-sum, normalization) is exact fp32.
"""
import numpy as np
from contextlib import ExitStack

import concourse.bass as bass
import concourse.tile as tile
from concourse import bacc, mybir
from concourse.bass_utils import run_bass_kernel_spmd

F32 = mybir.dt.float32
F32R = mybir.dt.float32r

# Problem constants (hardcoded per contract)
B = 8
S = 1024
MD = 768          # model dim
H = 12            # heads
DK = 64           # head dim
KC = MD // 128    # 6 contraction chunks of 128
ST = S // 128     # 8 s1 tiles
NPAIR = H // 2    # 6 head pairs
SCALE = DK ** -0.5

_BUILD_CACHE = {}


def build_nc():
    """Build the SPMD per-core kernel. Same program on all 8 cores."""
    nc = bacc.Bacc("TRN2", target_bir_lowering=False, debug=False, num_devices=B)

    # ---- DRAM I/O ----  (matmul operands declared fp32r; same bytes as fp32)
    xT_d = nc.dram_tensor("xT", [MD, S], F32R, kind="ExternalInput")
    w_qk_d = nc.dram_tensor("w_qk", [H, 128, KC, 128], F32R, kind="ExternalInput")
    b_qk_d = nc.dram_tensor("b_qk", [128, H], F32, kind="ExternalInput")
    w_v_d = nc.dram_tensor("w_v", [KC, 128, MD], F32R, kind="ExternalInput")
    b_v_d = nc.dram_tensor("b_v", [128, MD], F32, kind="ExternalInput")
    w_out_d = nc.dram_tensor("w_out", [KC, 128, MD], F32R, kind="ExternalInput")
    b_out_d = nc.dram_tensor("b_out", [128, MD], F32, kind="ExternalInput")
    ident_d = nc.dram_tensor("ident", [128, 128], F32R, kind="ExternalInput")

    out_d = nc.dram_tensor("out", [S, MD], F32, kind="ExternalOutput")
    wts_d = nc.dram_tensor("weights", [H, S, S], F32R, kind="ExternalOutput")

    with tile.TileContext(nc) as tc:
        with ExitStack() as ctx:
            consts = ctx.enter_context(tc.tile_pool(name="consts", bufs=1))
            qkp = ctx.enter_context(tc.tile_pool(name="qkp", bufs=2))
            slabp = ctx.enter_context(tc.tile_pool(name="slabp", bufs=2))
            vp = ctx.enter_context(tc.tile_pool(name="vp", bufs=1))
            wtp = ctx.enter_context(tc.tile_pool(name="wtp", bufs=1))
            work = ctx.enter_context(tc.tile_pool(name="work", bufs=2))
            scal = ctx.enter_context(tc.tile_pool(name="scal", bufs=4))
            atp = ctx.enter_context(tc.tile_pool(name="atp", bufs=1))

            ps_big = ctx.enter_context(tc.tile_pool(name="ps_big", bufs=2, space="PSUM"))
            ps_t = ctx.enter_context(tc.tile_pool(name="ps_t", bufs=2, space="PSUM"))

            # ---- constant loads ----
            xT = [consts.tile([128, S], F32R, tag=f"xT{k}", name=f"xT{k}") for k in range(KC)]
            for k in range(KC):
                nc.sync.dma_start(out=xT[k], in_=xT_d[128 * k:128 * (k + 1), :])
            w_v = [consts.tile([128, MD], F32R, tag=f"wv{k}", name=f"wv{k}") for k in range(KC)]
            for k in range(KC):
                nc.sync.dma_start(out=w_v[k], in_=w_v_d[k])
            w_out = [consts.tile([128, MD], F32R, tag=f"wo{k}", name=f"wo{k}") for k in range(KC)]
            for k in range(KC):
                nc.sync.dma_start(out=w_out[k], in_=w_out_d[k])
            b_qk = consts.tile([128, H], F32, tag="bqk")
            nc.sync.dma_start(out=b_qk, in_=b_qk_d[:, :])
            b_v = consts.tile([128, MD], F32, tag="bv")
            nc.sync.dma_start(out=b_v, in_=b_v_d[:, :])
            b_out = consts.tile([128, MD], F32, tag="bout")
            nc.sync.dma_start(out=b_out, in_=b_out_d[:, :])
            ident = consts.tile([128, 128], F32R, tag="id")
            nc.sync.dma_start(out=ident, in_=ident_d[:, :])

            NSL = [(0, 512), (512, MD - 512)]  # N slices for MD-wide matmuls

            # ---- V projection: v[s2, (h d)] = xT.T @ W_v + b_v ----
            v_t = [vp.tile([128, MD], F32R, tag=f"v{mt}", name=f"v{mt}") for mt in range(ST)]
            for mt in range(ST):
                ps = ps_t.tile([128, 1024], F32, tag="pt", name=f"vps{mt}")
                for k in range(KC):
                    for (n0, nn) in NSL:
                        nc.tensor.matmul(
                            ps[:, n0:n0 + nn],
                            xT[k][:, 128 * mt:128 * (mt + 1)],
                            w_v[k][:, n0:n0 + nn],
                            start=(k == 0), stop=(k == KC - 1),
                        )
                nc.vector.tensor_add(v_t[mt], ps[:, :MD], b_v)

            # wT chunk layout: chunk j occupies cols [1024j, 1024(j+1))
            wT = wtp.tile([128, ST * 1024], F32R, tag="wT")
            wT_v = wT.rearrange("p (j q) -> p j q", j=ST)

            # ---- per head-pair ----
            at_all = []
            for c in range(NPAIR):
                slabq = slabp.tile([128, MD], F32R, tag="slabq")
                nc.sync.dma_start(out=slabq, in_=w_qk_d[c].rearrange("p k c -> p (k c)"))
                slabk = slabp.tile([128, MD], F32R, tag="slabk")
                nc.sync.dma_start(out=slabk, in_=w_qk_d[NPAIR + c].rearrange("p k c -> p (k c)"))

                qt = qkp.tile([128, S], F32R, tag="qt")
                kt = qkp.tile([128, S], F32R, tag="kt")
                for dst, slab, bi in ((qt, slabq, c), (kt, slabk, NPAIR + c)):
                    ps = ps_big.tile([128, 1024], F32, tag="big", name="qkps")
                    for k in range(KC):
                        for nt in range(2):
                            nc.tensor.matmul(
                                ps[:, 512 * nt:512 * (nt + 1)],
                                slab[:, 128 * k:128 * (k + 1)],
                                xT[k][:, 512 * nt:512 * (nt + 1)],
                                start=(k == 0), stop=(k == KC - 1),
                            )
                    nc.scalar.add(dst, ps, b_qk[:, bi:bi + 1])

                at = atp.tile([128, S], F32R, tag=f"at{c}", name=f"at{c}")
                for r in range(2):
                    h = 2 * c + r
                    for s1t in range(ST):
                        sc = ps_big.tile([128, 1024], F32, tag="big", name="scps")
                        for nt in range(2):
                            nc.tensor.matmul(
                                sc[:, 512 * nt:512 * (nt + 1)],
                                qt[64 * r:64 * (r + 1), 128 * s1t:128 * (s1t + 1)],
                                kt[64 * r:64 * (r + 1), 512 * nt:512 * (nt + 1)],
                                start=True, stop=True,
                            )
                        e = work.tile([128, 1024], F32, tag="e")
                        ssum = scal.tile([128, 1], F32, tag="ss")
                        nc.scalar.activation(
                            e, sc, mybir.ActivationFunctionType.Exp,
                            bias=0.0, scale=SCALE, accum_out=ssum,
                        )
                        rcp = scal.tile([128, 1], F32, tag="rc")
                        nc.vector.reciprocal(rcp, ssum)
                        wtile = work.tile([128, 1024], F32R, tag="w")
                        nc.vector.tensor_scalar_mul(wtile, e, rcp)
                        nc.sync.dma_start(
                            out=wts_d[h, 128 * s1t:128 * (s1t + 1), :], in_=wtile
                        )
                        tp = ps_t.tile([128, 1024], F32R, tag="pt", name="tps")
                        for j in range(ST):
                            nc.tensor.transpose(
                                tp[:, 128 * j:128 * (j + 1)],
                                wtile[:, 128 * j:128 * (j + 1)],
                                ident,
                            )
                        tp_v = tp.rearrange("p (j q) -> p j q", j=ST)
                        JS = 5  # DVE takes chunks 0..4, ACT takes 5..7
                        nc.vector.tensor_copy(
                            wT_v[:, :JS, 128 * s1t:128 * (s1t + 1)],
                            tp_v[:, :JS, :],
                        )
                        nc.scalar.copy(
                            wT_v[:, JS:, 128 * s1t:128 * (s1t + 1)],
                            tp_v[:, JS:, :],
                        )
                    # attention: attnT[d, s1] for head h (fp32r needs dst part 0)
                    pa = ps_t.tile([64, 1024], F32, tag="pt", name=f"pa{h}")
                    for nt in range(2):
                        for j in range(ST):
                            nc.tensor.matmul(
                                pa[:, 512 * nt:512 * (nt + 1)],
                                v_t[j][:, 64 * h:64 * (h + 1)],
                                wT_v[:, j, 512 * nt:512 * (nt + 1)],
                                start=(j == 0), stop=(j == ST - 1),
                            )
                    nc.scalar.copy(at[64 * r:64 * (r + 1), :], pa)
                at_all.append(at)

            # ---- output projection: out[s1, m] = attnT.T @ W_out + b_out ----
            for s1t in range(ST):
                ps = ps_t.tile([128, 1024], F32, tag="pt", name=f"ops{s1t}")
                for cc in range(KC):
                    for (n0, nn) in NSL:
                        nc.tensor.matmul(
                            ps[:, n0:n0 + nn],
                            at_all[cc][:, 128 * s1t:128 * (s1t + 1)],
                            w_out[cc][:, n0:n0 + nn],
                            start=(cc == 0), stop=(cc == KC - 1),
                        )
                osb = work.tile([128, MD], F32, tag="osb")
                nc.vector.tensor_add(osb, ps[:, :MD], b_out)
                nc.sync.dma_start(
                    out=out_d[128 * s1t:128 * (s1t + 1), :], in_=osb
                )

    nc.compile()
    return nc


def prep_inputs(x, W_qkv, b_qkv, W_out, b_out):
    """Host-side resharding/reordering. Returns per-core input maps."""
    x = np.asarray(x, dtype=np.float32)
    W_qkv = np.asarray(W_qkv, dtype=np.float32)
    b_qkv = np.asarray(b_qkv, dtype=np.float32)
    W_out = np.asarray(W_out, dtype=np.float32)
    b_out = np.asarray(b_out, dtype=np.float32)

    # original col index = h*(DK*3) + d*3 + comp
    cols = np.arange(H * DK * 3).reshape(H, DK, 3)
    idx_q = cols[:, :, 0].reshape(-1)     # (h, d) order
    idx_k = cols[:, :, 1].reshape(-1)
    idx_v = cols[:, :, 2].reshape(-1)

    W_QK = np.concatenate([W_qkv[:, idx_q], W_qkv[:, idx_k]], axis=1)  # [768, 1536]
    b_QK = np.concatenate([b_qkv[idx_q], b_qkv[idx_k]])                # [1536]
    W_V = W_qkv[:, idx_v]                                              # [768, 768]
    b_V = b_qkv[idx_v]

    # w_qk tiled: w_qk[m, p, k, cc] = W_QK[128k + p, 128m + cc]
    w_qk = np.ascontiguousarray(
        W_QK.reshape(KC, 128, H, 128).transpose(2, 1, 0, 3)
    )
    b_qk = np.ascontiguousarray(b_QK.reshape(H, 128).T)               # [128, 12]
    w_v = np.ascontiguousarray(W_V.reshape(KC, 128, MD))
    b_v = np.ascontiguousarray(np.broadcast_to(b_V, (128, MD)))
    w_out_t = np.ascontiguousarray(W_out.reshape(KC, 128, MD))
    b_out_b = np.ascontiguousarray(np.broadcast_to(b_out, (128, MD)))
    ident = np.eye(128, dtype=np.float32)

    shared = {
        "w_qk": w_qk, "b_qk": b_qk, "w_v": w_v, "b_v": b_v,
        "w_out": w_out_t, "b_out": b_out_b, "ident": ident,
    }
    in_maps = []
    for b in range(B):
        m = dict(shared)
        m["xT"] = np.ascontiguousarray(x[b].T)
        in_maps.append(m)
    return in_maps


def _get_nc():
    if "nc" not in _BUILD_CACHE:
        _BUILD_CACHE["nc"] = build_nc()
    return _BUILD_CACHE["nc"]


def run(inputs, trace=False, **kw):
    nc = _get_nc()
    in_maps = prep_inputs(**inputs)
    res = run_bass_kernel_spmd(nc, in_maps, core_ids=list(range(B)), trace=trace, **kw)
    out = np.stack([res.results[b]["out"] for b in range(B)])
    weights = np.stack([res.results[b]["weights"] for b in range(B)])
    return (out, weights), res


def kernel(x, W_qkv, b_qkv, W_out, b_out):
    (out, weights), _ = run(
        dict(x=x, W_qkv=W_qkv, b_qkv=b_qkv, W_out=W_out, b_out=b_out)
    )
    return out, weights


# revision 20
# speedup vs baseline: 1.0863x; 1.0326x over previous
"""Bass/Trainium2 kernel for nn_MultiHeadAttention (B=8, S=1024, H=12, D=64, model=768).

Sharding: pure data parallel over batch — core b computes batch element b.
Returns (out, weights) matching the reference.

Matmuls run in fp32r (full PE rate, ~1.6e-4 matmul relerr vs fp32's 4x slowdown).
Softmax (exp, row-sum, normalization) is exact fp32.
"""
import numpy as np
from contextlib import ExitStack

import concourse.bass as bass
import concourse.tile as tile
from concourse import bacc, mybir
from concourse.bass_utils import run_bass_kernel_spmd

F32 = mybir.dt.float32
F32R = mybir.dt.float32r

# Problem constants (hardcoded per contract)
B = 8
S = 1024
MD = 768          # model dim
H = 12            # heads
DK = 64           # head dim
KC = MD // 128    # 6 contraction chunks of 128
ST = S // 128     # 8 s1 tiles
NPAIR = H // 2    # 6 head pairs
SCALE = DK ** -0.5

_BUILD_CACHE = {}


def build_nc():
    """Build the SPMD per-core kernel. Same program on all 8 cores."""
    nc = bacc.Bacc("TRN2", target_bir_lowering=False, debug=False, num_devices=B)

    # ---- DRAM I/O ----  (matmul operands declared fp32r; same bytes as fp32)
    xT_d = nc.dram_tensor("xT", [MD, S], F32R, kind="ExternalInput")
    w_qk_d = nc.dram_tensor("w_qk", [H, 128, KC, 128], F32R, kind="ExternalInput")
    b_qk_d = nc.dram_tensor("b_qk", [128, H], F32, kind="ExternalInput")
    w_v_d = nc.dram_tensor("w_v", [KC, 128, MD], F32R, kind="ExternalInput")
    b_v_d = nc.dram_tensor("b_v", [128, MD], F32, kind="ExternalInput")
    w_out_d = nc.dram_tensor("w_out", [KC, 128, MD], F32R, kind="ExternalInput")
    b_out_d = nc.dram_tensor("b_out", [128, MD], F32, kind="ExternalInput")
    ident_d = nc.dram_tensor("ident", [128, 128], F32R, kind="ExternalInput")

    out_d = nc.dram_tensor("out", [S, MD], F32, kind="ExternalOutput")
    wts_d = nc.dram_tensor("weights", [H, S, S], F32R, kind="ExternalOutput")

    with tile.TileContext(nc) as tc:
        with ExitStack() as ctx:
            consts = ctx.enter_context(tc.tile_pool(name="consts", bufs=1))
            qkp = ctx.enter_context(tc.tile_pool(name="qkp", bufs=2))
            slabp = ctx.enter_context(tc.tile_pool(name="slabp", bufs=2))
            vp = ctx.enter_context(tc.tile_pool(name="vp", bufs=1))
            wtp = ctx.enter_context(tc.tile_pool(name="wtp", bufs=1))
            work = ctx.enter_context(tc.tile_pool(name="work", bufs=2))
            scal = ctx.enter_context(tc.tile_pool(name="scal", bufs=4))
            atp = ctx.enter_context(tc.tile_pool(name="atp", bufs=1))

            ps_big = ctx.enter_context(tc.tile_pool(name="ps_big", bufs=2, space="PSUM"))
            ps_t = ctx.enter_context(tc.tile_pool(name="ps_t", bufs=2, space="PSUM"))

            # ---- constant loads ----
            xT = [consts.tile([128, S], F32R, tag=f"xT{k}", name=f"xT{k}") for k in range(KC)]
            for k in range(KC):
                nc.sync.dma_start(out=xT[k], in_=xT_d[128 * k:128 * (k + 1), :])
            w_v = [consts.tile([128, MD], F32R, tag=f"wv{k}", name=f"wv{k}") for k in range(KC)]
            for k in range(KC):
                nc.sync.dma_start(out=w_v[k], in_=w_v_d[k])
            b_qk = consts.tile([128, H], F32, tag="bqk")
            nc.sync.dma_start(out=b_qk, in_=b_qk_d[:, :])
            b_v = consts.tile([128, MD], F32, tag="bv")
            nc.sync.dma_start(out=b_v, in_=b_v_d[:, :])
            b_out = consts.tile([128, MD], F32, tag="bout")
            nc.sync.dma_start(out=b_out, in_=b_out_d[:, :])
            ident = consts.tile([128, 128], F32R, tag="id")
            nc.sync.dma_start(out=ident, in_=ident_d[:, :])

            NSL = [(0, 512), (512, MD - 512)]  # N slices for MD-wide matmuls

            # ---- pair-0 slabs prefetch (small, unblocks first scores early) ----
            slabq0 = slabp.tile([128, MD], F32R, tag="slabq", name="slabq0")
            nc.sync.dma_start(out=slabq0, in_=w_qk_d[0].rearrange("p k c -> p (k c)"))
            slabk0 = slabp.tile([128, MD], F32R, tag="slabk", name="slabk0")
            nc.sync.dma_start(out=slabk0, in_=w_qk_d[NPAIR].rearrange("p k c -> p (k c)"))

            # ---- V projection: v[s2, (h d)] = xT.T @ W_v + b_v ----
            v_t = [vp.tile([128, MD], F32R, tag=f"v{mt}", name=f"v{mt}") for mt in range(ST)]
            for mt in range(ST):
                ps = ps_t.tile([128, 1024], F32, tag="pt", name=f"vps{mt}")
                for k in range(KC):
                    for (n0, nn) in NSL:
                        nc.tensor.matmul(
                            ps[:, n0:n0 + nn],
                            xT[k][:, 128 * mt:128 * (mt + 1)],
                            w_v[k][:, n0:n0 + nn],
                            start=(k == 0), stop=(k == KC - 1),
                        )
                nc.vector.tensor_add(v_t[mt], ps[:, :MD], b_v)

            w_out = [consts.tile([128, MD], F32R, tag=f"wo{k}", name=f"wo{k}") for k in range(KC)]
            for k in range(KC):
                nc.sync.dma_start(out=w_out[k], in_=w_out_d[k])

            # wT chunk layout: chunk j occupies cols [1024j, 1024(j+1))
            wT = wtp.tile([128, ST * 1024], F32R, tag="wT")
            wT_v = wT.rearrange("p (j q) -> p j q", j=ST)

            # ---- per head-pair ----
            at_all = []
            for c in range(NPAIR):
                if c == 0:
                    slabq, slabk = slabq0, slabk0
                else:
                    slabq = slabp.tile([128, MD], F32R, tag="slabq")
                    nc.sync.dma_start(out=slabq, in_=w_qk_d[c].rearrange("p k c -> p (k c)"))
                    slabk = slabp.tile([128, MD], F32R, tag="slabk")
                    nc.sync.dma_start(out=slabk, in_=w_qk_d[NPAIR + c].rearrange("p k c -> p (k c)"))

                qt = qkp.tile([128, S], F32R, tag="qt")
                kt = qkp.tile([128, S], F32R, tag="kt")
                for dst, slab, bi in ((qt, slabq, c), (kt, slabk, NPAIR + c)):
                    ps = ps_big.tile([128, 1024], F32, tag="big", name="qkps")
                    for k in range(KC):
                        for nt in range(2):
                            nc.tensor.matmul(
                                ps[:, 512 * nt:512 * (nt + 1)],
                                slab[:, 128 * k:128 * (k + 1)],
                                xT[k][:, 512 * nt:512 * (nt + 1)],
                                start=(k == 0), stop=(k == KC - 1),
                            )
                    nc.scalar.add(dst, ps, b_qk[:, bi:bi + 1])

                at = atp.tile([128, S], F32R, tag=f"at{c}", name=f"at{c}")
                for r in range(2):
                    h = 2 * c + r
                    for s1t in range(ST):
                        sc = ps_big.tile([128, 1024], F32, tag="big", name="scps")
                        for nt in range(2):
                            nc.tensor.matmul(
                                sc[:, 512 * nt:512 * (nt + 1)],
                                qt[64 * r:64 * (r + 1), 128 * s1t:128 * (s1t + 1)],
                                kt[64 * r:64 * (r + 1), 512 * nt:512 * (nt + 1)],
                                start=True, stop=True,
                            )
                        e = work.tile([128, 1024], F32, tag="e")
                        ssum = scal.tile([128, 1], F32, tag="ss")
                        nc.scalar.activation(
                            e, sc, mybir.ActivationFunctionType.Exp,
                            bias=0.0, scale=SCALE, accum_out=ssum,
                        )
                        rcp = scal.tile([128, 1], F32, tag="rc")
                        nc.vector.reciprocal(rcp, ssum)
                        wtile = work.tile([128, 1024], F32R, tag="w")
                        nc.vector.tensor_scalar_mul(wtile, e, rcp)
                        nc.sync.dma_start(
                            out=wts_d[h, 128 * s1t:128 * (s1t + 1), :], in_=wtile
                        )
                        tp = ps_t.tile([128, 1024], F32R, tag="pt", name="tps")
                        for j in range(ST):
                            nc.tensor.transpose(
                                tp[:, 128 * j:128 * (j + 1)],
                                wtile[:, 128 * j:128 * (j + 1)],
                                ident,
                            )
                        tp_v = tp.rearrange("p (j q) -> p j q", j=ST)
                        JS = 5  # DVE takes chunks 0..4, ACT takes 5..7
                        nc.vector.tensor_copy(
                            wT_v[:, :JS, 128 * s1t:128 * (s1t + 1)],
                            tp_v[:, :JS, :],
                        )
                        nc.scalar.copy(
                            wT_v[:, JS:, 128 * s1t:128 * (s1t + 1)],
                            tp_v[:, JS:, :],
                        )
                    # attention: attnT[d, s1] for head h (fp32r needs dst part 0)
                    pa = ps_t.tile([64, 1024], F32, tag="pt", name=f"pa{h}")
                    for nt in range(2):
                        for j in range(ST):
                            nc.tensor.matmul(
                                pa[:, 512 * nt:512 * (nt + 1)],
                                v_t[j][:, 64 * h:64 * (h + 1)],
                                wT_v[:, j, 512 * nt:512 * (nt + 1)],
                                start=(j == 0), stop=(j == ST - 1),
                            )
                    nc.scalar.copy(at[64 * r:64 * (r + 1), :], pa)
                at_all.append(at)

            # ---- output projection: out[s1, m] = attnT.T @ W_out + b_out ----
            for s1t in range(ST):
                ps = ps_t.tile([128, 1024], F32, tag="pt", name=f"ops{s1t}")
                for cc in range(KC):
                    for (n0, nn) in NSL:
                        nc.tensor.matmul(
                            ps[:, n0:n0 + nn],
                            at_all[cc][:, 128 * s1t:128 * (s1t + 1)],
                            w_out[cc][:, n0:n0 + nn],
                            start=(cc == 0), stop=(cc == KC - 1),
                        )
                osb = work.tile([128, MD], F32, tag="osb")
                nc.vector.tensor_add(osb, ps[:, :MD], b_out)
                nc.sync.dma_start(
                    out=out_d[128 * s1t:128 * (s1t + 1), :], in_=osb
                )

    nc.compile()
    return nc


def prep_inputs(x, W_qkv, b_qkv, W_out, b_out):
    """Host-side resharding/reordering. Returns per-core input maps."""
    x = np.asarray(x, dtype=np.float32)
    W_qkv = np.asarray(W_qkv, dtype=np.float32)
    b_qkv = np.asarray(b_qkv, dtype=np.float32)
    W_out = np.asarray(W_out, dtype=np.float32)
    b_out = np.asarray(b_out, dtype=np.float32)

    # original col index = h*(DK*3) + d*3 + comp
    cols = np.arange(H * DK * 3).reshape(H, DK, 3)
    idx_q = cols[:, :, 0].reshape(-1)     # (h, d) order
    idx_k = cols[:, :, 1].reshape(-1)
    idx_v = cols[:, :, 2].reshape(-1)

    W_QK = np.concatenate([W_qkv[:, idx_q], W_qkv[:, idx_k]], axis=1)  # [768, 1536]
    b_QK = np.concatenate([b_qkv[idx_q], b_qkv[idx_k]])                # [1536]
    W_V = W_qkv[:, idx_v]                                              # [768, 768]
    b_V = b_qkv[idx_v]

    # w_qk tiled: w_qk[m, p, k, cc] = W_QK[128k + p, 128m + cc]
    w_qk = np.ascontiguousarray(
        W_QK.reshape(KC, 128, H, 128).transpose(2, 1, 0, 3)
    )
    b_qk = np.ascontiguousarray(b_QK.reshape(H, 128).T)               # [128, 12]
    w_v = np.ascontiguousarray(W_V.reshape(KC, 128, MD))
    b_v = np.ascontiguousarray(np.broadcast_to(b_V, (128, MD)))
    w_out_t = np.ascontiguousarray(W_out.reshape(KC, 128, MD))
    b_out_b = np.ascontiguousarray(np.broadcast_to(b_out, (128, MD)))
    ident = np.eye(128, dtype=np.float32)

    shared = {
        "w_qk": w_qk, "b_qk": b_qk, "w_v": w_v, "b_v": b_v,
        "w_out": w_out_t, "b_out": b_out_b, "ident": ident,
    }
    in_maps = []
    for b in range(B):
        m = dict(shared)
        m["xT"] = np.ascontiguousarray(x[b].T)
        in_maps.append(m)
    return in_maps


def _get_nc():
    if "nc" not in _BUILD_CACHE:
        _BUILD_CACHE["nc"] = build_nc()
    return _BUILD_CACHE["nc"]


def run(inputs, trace=False, **kw):
    nc = _get_nc()
    in_maps = prep_inputs(**inputs)
    res = run_bass_kernel_spmd(nc, in_maps, core_ids=list(range(B)), trace=trace, **kw)
    out = np.stack([res.results[b]["out"] for b in range(B)])
    weights = np.stack([res.results[b]["weights"] for b in range(B)])
    return (out, weights), res


def kernel(x, W_qkv, b_qkv, W_out, b_out):
    (out, weights), _ = run(
        dict(x=x, W_qkv=W_qkv, b_qkv=b_qkv, W_out=W_out, b_out=b_out)
    )
    return out, weights


# revision 21
# speedup vs baseline: 1.1026x; 1.0151x over previous
"""Bass/Trainium2 kernel for nn_MultiHeadAttention (B=8, S=1024, H=12, D=64, model=768).

Sharding: pure data parallel over batch — core b computes batch element b.
Returns (out, weights) matching the reference.

Matmuls run in fp32r (full PE rate, ~1.6e-4 matmul relerr vs fp32's 4x slowdown).
Softmax (exp, row-sum, normalization) is exact fp32.
"""
import numpy as np
from contextlib import ExitStack

import concourse.bass as bass
import concourse.tile as tile
from concourse import bacc, mybir
from concourse.bass_utils import run_bass_kernel_spmd

F32 = mybir.dt.float32
F32R = mybir.dt.float32r

# Problem constants (hardcoded per contract)
B = 8
S = 1024
MD = 768          # model dim
H = 12            # heads
DK = 64           # head dim
KC = MD // 128    # 6 contraction chunks of 128
ST = S // 128     # 8 s1 tiles
NPAIR = H // 2    # 6 head pairs
SCALE = DK ** -0.5

_BUILD_CACHE = {}


def build_nc():
    """Build the SPMD per-core kernel. Same program on all 8 cores."""
    nc = bacc.Bacc("TRN2", target_bir_lowering=False, debug=False, num_devices=B)

    # ---- DRAM I/O ----  (matmul operands declared fp32r; same bytes as fp32)
    xT_d = nc.dram_tensor("xT", [MD, S], F32R, kind="ExternalInput")
    w_qk_d = nc.dram_tensor("w_qk", [H, 128, KC, 128], F32R, kind="ExternalInput")
    b_qk_d = nc.dram_tensor("b_qk", [128, H], F32, kind="ExternalInput")
    w_v_d = nc.dram_tensor("w_v", [KC, 128, MD], F32R, kind="ExternalInput")
    b_v_d = nc.dram_tensor("b_v", [128, MD], F32, kind="ExternalInput")
    w_out_d = nc.dram_tensor("w_out", [KC, 128, MD], F32R, kind="ExternalInput")
    b_out_d = nc.dram_tensor("b_out", [128, MD], F32, kind="ExternalInput")
    ident_d = nc.dram_tensor("ident", [128, 128], F32R, kind="ExternalInput")

    out_d = nc.dram_tensor("out", [S, MD], F32, kind="ExternalOutput")
    wts_d = nc.dram_tensor("weights", [H, S, S], F32R, kind="ExternalOutput")

    with tile.TileContext(nc) as tc:
        with ExitStack() as ctx:
            consts = ctx.enter_context(tc.tile_pool(name="consts", bufs=1))
            qkp = ctx.enter_context(tc.tile_pool(name="qkp", bufs=2))
            slabp = ctx.enter_context(tc.tile_pool(name="slabp", bufs=2))
            vp = ctx.enter_context(tc.tile_pool(name="vp", bufs=1))
            wtp = ctx.enter_context(tc.tile_pool(name="wtp", bufs=1))
            work = ctx.enter_context(tc.tile_pool(name="work", bufs=2))
            scal = ctx.enter_context(tc.tile_pool(name="scal", bufs=4))
            atp = ctx.enter_context(tc.tile_pool(name="atp", bufs=1))

            ps_big = ctx.enter_context(tc.tile_pool(name="ps_big", bufs=2, space="PSUM"))
            ps_t = ctx.enter_context(tc.tile_pool(name="ps_t", bufs=2, space="PSUM"))

            # ---- pair-0 slabs first: unblocks the first QK projection ASAP ----
            slabq0 = slabp.tile([128, MD], F32R, tag="slabq", name="slabq0")
            nc.sync.dma_start(out=slabq0, in_=w_qk_d[0].rearrange("p k c -> p (k c)"))
            slabk0 = slabp.tile([128, MD], F32R, tag="slabk", name="slabk0")
            nc.sync.dma_start(out=slabk0, in_=w_qk_d[NPAIR].rearrange("p k c -> p (k c)"))

            # ---- constant loads ----
            xT = [consts.tile([128, S], F32R, tag=f"xT{k}", name=f"xT{k}") for k in range(KC)]
            for k in range(KC):
                nc.sync.dma_start(out=xT[k], in_=xT_d[128 * k:128 * (k + 1), :])
            w_v = [consts.tile([128, MD], F32R, tag=f"wv{k}", name=f"wv{k}") for k in range(KC)]
            for k in range(KC):
                nc.sync.dma_start(out=w_v[k], in_=w_v_d[k])
            b_qk = consts.tile([128, H], F32, tag="bqk")
            nc.sync.dma_start(out=b_qk, in_=b_qk_d[:, :])
            b_v = consts.tile([128, MD], F32, tag="bv")
            nc.sync.dma_start(out=b_v, in_=b_v_d[:, :])
            b_out = consts.tile([128, MD], F32, tag="bout")
            nc.sync.dma_start(out=b_out, in_=b_out_d[:, :])
            ident = consts.tile([128, 128], F32R, tag="id")
            nc.sync.dma_start(out=ident, in_=ident_d[:, :])

            NSL = [(0, 512), (512, MD - 512)]  # N slices for MD-wide matmuls

            # ---- pair-0 QK projection (hoisted: starts as soon as slab0 + xT land) ----
            qt0 = qkp.tile([128, S], F32R, tag="qt", name="qt0")
            kt0 = qkp.tile([128, S], F32R, tag="kt", name="kt0")
            for dst, slab, bi in ((qt0, slabq0, 0), (kt0, slabk0, NPAIR)):
                qps = ps_big.tile([128, 1024], F32, tag="big", name="qkps0")
                for k in range(KC):
                    for nt in range(2):
                        nc.tensor.matmul(
                            qps[:, 512 * nt:512 * (nt + 1)],
                            slab[:, 128 * k:128 * (k + 1)],
                            xT[k][:, 512 * nt:512 * (nt + 1)],
                            start=(k == 0), stop=(k == KC - 1),
                        )
                nc.scalar.add(dst, qps, b_qk[:, bi:bi + 1])

            # ---- V projection: v[s2, (h d)] = xT.T @ W_v + b_v ----
            v_t = [vp.tile([128, MD], F32R, tag=f"v{mt}", name=f"v{mt}") for mt in range(ST)]
            for mt in range(ST):
                ps = ps_t.tile([128, 1024], F32, tag="pt", name=f"vps{mt}")
                for k in range(KC):
                    for (n0, nn) in NSL:
                        nc.tensor.matmul(
                            ps[:, n0:n0 + nn],
                            xT[k][:, 128 * mt:128 * (mt + 1)],
                            w_v[k][:, n0:n0 + nn],
                            start=(k == 0), stop=(k == KC - 1),
                        )
                nc.vector.tensor_add(v_t[mt], ps[:, :MD], b_v)

            w_out = [consts.tile([128, MD], F32R, tag=f"wo{k}", name=f"wo{k}") for k in range(KC)]
            for k in range(KC):
                nc.sync.dma_start(out=w_out[k], in_=w_out_d[k])

            # wT chunk layout: chunk j occupies cols [1024j, 1024(j+1))
            wT = wtp.tile([128, ST * 1024], F32R, tag="wT")
            wT_v = wT.rearrange("p (j q) -> p j q", j=ST)

            # ---- per head-pair ----
            at_all = []
            for c in range(NPAIR):
                if c == 0:
                    slabq, slabk = None, None
                else:
                    slabq = slabp.tile([128, MD], F32R, tag="slabq")
                    nc.sync.dma_start(out=slabq, in_=w_qk_d[c].rearrange("p k c -> p (k c)"))
                    slabk = slabp.tile([128, MD], F32R, tag="slabk")
                    nc.sync.dma_start(out=slabk, in_=w_qk_d[NPAIR + c].rearrange("p k c -> p (k c)"))

                if c == 0:
                    qt, kt = qt0, kt0
                else:
                    qt = qkp.tile([128, S], F32R, tag="qt")
                    kt = qkp.tile([128, S], F32R, tag="kt")
                    for dst, slab, bi in ((qt, slabq, c), (kt, slabk, NPAIR + c)):
                        ps = ps_big.tile([128, 1024], F32, tag="big", name="qkps")
                        for k in range(KC):
                            for nt in range(2):
                                nc.tensor.matmul(
                                    ps[:, 512 * nt:512 * (nt + 1)],
                                    slab[:, 128 * k:128 * (k + 1)],
                                    xT[k][:, 512 * nt:512 * (nt + 1)],
                                    start=(k == 0), stop=(k == KC - 1),
                                )
                        nc.scalar.add(dst, ps, b_qk[:, bi:bi + 1])

                at = atp.tile([128, S], F32R, tag=f"at{c}", name=f"at{c}")
                for r in range(2):
                    h = 2 * c + r
                    for s1t in range(ST):
                        sc = ps_big.tile([128, 1024], F32, tag="big", name="scps")
                        for nt in range(2):
                            nc.tensor.matmul(
                                sc[:, 512 * nt:512 * (nt + 1)],
                                qt[64 * r:64 * (r + 1), 128 * s1t:128 * (s1t + 1)],
                                kt[64 * r:64 * (r + 1), 512 * nt:512 * (nt + 1)],
                                start=True, stop=True,
                            )
                        e = work.tile([128, 1024], F32, tag="e")
                        ssum = scal.tile([128, 1], F32, tag="ss")
                        nc.scalar.activation(
                            e, sc, mybir.ActivationFunctionType.Exp,
                            bias=0.0, scale=SCALE, accum_out=ssum,
                        )
                        rcp = scal.tile([128, 1], F32, tag="rc")
                        nc.vector.reciprocal(rcp, ssum)
                        wtile = work.tile([128, 1024], F32R, tag="w")
                        nc.vector.tensor_scalar_mul(wtile, e, rcp)
                        nc.sync.dma_start(
                            out=wts_d[h, 128 * s1t:128 * (s1t + 1), :], in_=wtile
                        )
                        tp = ps_t.tile([128, 1024], F32R, tag="pt", name="tps")
                        for j in range(ST):
                            nc.tensor.transpose(
                                tp[:, 128 * j:128 * (j + 1)],
                                wtile[:, 128 * j:128 * (j + 1)],
                                ident,
                            )
                        tp_v = tp.rearrange("p (j q) -> p j q", j=ST)
                        JS = 5  # DVE takes chunks 0..4, ACT takes 5..7
                        nc.vector.tensor_copy(
                            wT_v[:, :JS, 128 * s1t:128 * (s1t + 1)],
                            tp_v[:, :JS, :],
                        )
                        nc.scalar.copy(
                            wT_v[:, JS:, 128 * s1t:128 * (s1t + 1)],
                            tp_v[:, JS:, :],
                        )
                    # attention: attnT[d, s1] for head h (fp32r needs dst part 0)
                    pa = ps_t.tile([64, 1024], F32, tag="pt", name=f"pa{h}")
                    for nt in range(2):
                        for j in range(ST):
                            nc.tensor.matmul(
                                pa[:, 512 * nt:512 * (nt + 1)],
                                v_t[j][:, 64 * h:64 * (h + 1)],
                                wT_v[:, j, 512 * nt:512 * (nt + 1)],
                                start=(j == 0), stop=(j == ST - 1),
                            )
                    nc.scalar.copy(at[64 * r:64 * (r + 1), :], pa)
                at_all.append(at)

            # ---- output projection: out[s1, m] = attnT.T @ W_out + b_out ----
            for s1t in range(ST):
                ps = ps_t.tile([128, 1024], F32, tag="pt", name=f"ops{s1t}")
                for cc in range(KC):
                    for (n0, nn) in NSL:
                        nc.tensor.matmul(
                            ps[:, n0:n0 + nn],
                            at_all[cc][:, 128 * s1t:128 * (s1t + 1)],
                            w_out[cc][:, n0:n0 + nn],
                            start=(cc == 0), stop=(cc == KC - 1),
                        )
                osb = work.tile([128, MD], F32, tag="osb")
                nc.vector.tensor_add(osb, ps[:, :MD], b_out)
                nc.sync.dma_start(
                    out=out_d[128 * s1t:128 * (s1t + 1), :], in_=osb
                )

    nc.compile()
    return nc


def prep_inputs(x, W_qkv, b_qkv, W_out, b_out):
    """Host-side resharding/reordering. Returns per-core input maps."""
    x = np.asarray(x, dtype=np.float32)
    W_qkv = np.asarray(W_qkv, dtype=np.float32)
    b_qkv = np.asarray(b_qkv, dtype=np.float32)
    W_out = np.asarray(W_out, dtype=np.float32)
    b_out = np.asarray(b_out, dtype=np.float32)

    # original col index = h*(DK*3) + d*3 + comp
    cols = np.arange(H * DK * 3).reshape(H, DK, 3)
    idx_q = cols[:, :, 0].reshape(-1)     # (h, d) order
    idx_k = cols[:, :, 1].reshape(-1)
    idx_v = cols[:, :, 2].reshape(-1)

    W_QK = np.concatenate([W_qkv[:, idx_q], W_qkv[:, idx_k]], axis=1)  # [768, 1536]
    b_QK = np.concatenate([b_qkv[idx_q], b_qkv[idx_k]])                # [1536]
    W_V = W_qkv[:, idx_v]                                              # [768, 768]
    b_V = b_qkv[idx_v]

    # w_qk tiled: w_qk[m, p, k, cc] = W_QK[128k + p, 128m + cc]
    w_qk = np.ascontiguousarray(
        W_QK.reshape(KC, 128, H, 128).transpose(2, 1, 0, 3)
    )
    b_qk = np.ascontiguousarray(b_QK.reshape(H, 128).T)               # [128, 12]
    w_v = np.ascontiguousarray(W_V.reshape(KC, 128, MD))
    b_v = np.ascontiguousarray(np.broadcast_to(b_V, (128, MD)))
    w_out_t = np.ascontiguousarray(W_out.reshape(KC, 128, MD))
    b_out_b = np.ascontiguousarray(np.broadcast_to(b_out, (128, MD)))
    ident = np.eye(128, dtype=np.float32)

    shared = {
        "w_qk": w_qk, "b_qk": b_qk, "w_v": w_v, "b_v": b_v,
        "w_out": w_out_t, "b_out": b_out_b, "ident": ident,
    }
    in_maps = []
    for b in range(B):
        m = dict(shared)
        m["xT"] = np.ascontiguousarray(x[b].T)
        in_maps.append(m)
    return in_maps


def _get_nc():
    if "nc" not in _BUILD_CACHE:
        _BUILD_CACHE["nc"] = build_nc()
    return _BUILD_CACHE["nc"]


def run(inputs, trace=False, **kw):
    nc = _get_nc()
    in_maps = prep_inputs(**inputs)
    res = run_bass_kernel_spmd(nc, in_maps, core_ids=list(range(B)), trace=trace, **kw)
    out = np.stack([res.results[b]["out"] for b in range(B)])
    weights = np.stack([res.results[b]["weights"] for b in range(B)])
    return (out, weights), res


def kernel(x, W_qkv, b_qkv, W_out, b_out):
    (out, weights), _ = run(
        dict(x=x, W_qkv=W_qkv, b_qkv=b_qkv, W_out=W_out, b_out=b_out)
    )
    return out, weights


# revision 22
# speedup vs baseline: 1.2363x; 1.1212x over previous
"""Bass/Trainium2 kernel for nn_MultiHeadAttention (B=8, S=1024, H=12, D=64, model=768).

Sharding: pure data parallel over batch — core b computes batch element b.
Returns (out, weights) matching the reference.

Matmuls run in fp32r (full PE rate, ~1.6e-4 matmul relerr vs fp32's 4x slowdown).
Softmax (exp, row-sum, normalization) is exact fp32.
"""
import numpy as np
from contextlib import ExitStack

import concourse.bass as bass
import concourse.tile as tile
from concourse import bacc, mybir
from concourse.bass_utils import run_bass_kernel_spmd

F32 = mybir.dt.float32
F32R = mybir.dt.float32r

# Problem constants (hardcoded per contract)
B = 8
S = 1024
MD = 768          # model dim
H = 12            # heads
DK = 64           # head dim
KC = MD // 128    # 6 contraction chunks of 128
ST = S // 128     # 8 s1 tiles
NPAIR = H // 2    # 6 head pairs
SCALE = DK ** -0.5

_BUILD_CACHE = {}


def build_nc():
    """Build the SPMD per-core kernel. Same program on all 8 cores."""
    nc = bacc.Bacc("TRN2", target_bir_lowering=False, debug=False, num_devices=B)

    # ---- DRAM I/O ----  (matmul operands declared fp32r; same bytes as fp32)
    xT_d = nc.dram_tensor("xT", [MD, S], F32R, kind="ExternalInput")
    w_qk_d = nc.dram_tensor("w_qk", [H, 128, KC, 128], F32R, kind="ExternalInput")
    b_qk_d = nc.dram_tensor("b_qk", [128, H], F32, kind="ExternalInput")
    w_v_d = nc.dram_tensor("w_v", [KC, 128, MD], F32R, kind="ExternalInput")
    b_v_d = nc.dram_tensor("b_v", [128, MD], F32, kind="ExternalInput")
    w_out_d = nc.dram_tensor("w_out", [KC, 128, MD], F32R, kind="ExternalInput")
    b_out_d = nc.dram_tensor("b_out", [128, MD], F32, kind="ExternalInput")
    ident_d = nc.dram_tensor("ident", [128, 128], F32R, kind="ExternalInput")

    out_d = nc.dram_tensor("out", [S, MD], F32, kind="ExternalOutput")
    wts_d = nc.dram_tensor("weights", [H, S, S], F32R, kind="ExternalOutput")

    with tile.TileContext(nc) as tc:
        with ExitStack() as ctx:
            consts = ctx.enter_context(tc.tile_pool(name="consts", bufs=1))
            qkp = ctx.enter_context(tc.tile_pool(name="qkp", bufs=2))
            slabp = ctx.enter_context(tc.tile_pool(name="slabp", bufs=2))
            vp = ctx.enter_context(tc.tile_pool(name="vp", bufs=1))
            wtp = ctx.enter_context(tc.tile_pool(name="wtp", bufs=1))
            work = ctx.enter_context(tc.tile_pool(name="work", bufs=2))
            scal = ctx.enter_context(tc.tile_pool(name="scal", bufs=4))
            atp = ctx.enter_context(tc.tile_pool(name="atp", bufs=1))

            ps_big = ctx.enter_context(tc.tile_pool(name="ps_big", bufs=2, space="PSUM"))
            ps_t = ctx.enter_context(tc.tile_pool(name="ps_t", bufs=2, space="PSUM"))

            # ---- pair-0 slabs first: unblocks the first QK projection ASAP ----
            slabq0 = slabp.tile([128, MD], F32R, tag="slabq", name="slabq0")
            nc.sync.dma_start(out=slabq0, in_=w_qk_d[0].rearrange("p k c -> p (k c)"))
            slabk0 = slabp.tile([128, MD], F32R, tag="slabk", name="slabk0")
            nc.sync.dma_start(out=slabk0, in_=w_qk_d[NPAIR].rearrange("p k c -> p (k c)"))

            # ---- constant loads ----
            xT = [consts.tile([128, S], F32R, tag=f"xT{k}", name=f"xT{k}") for k in range(KC)]
            for k in range(KC):
                nc.sync.dma_start(out=xT[k], in_=xT_d[128 * k:128 * (k + 1), :])
            w_v = [consts.tile([128, MD], F32R, tag=f"wv{k}", name=f"wv{k}") for k in range(KC)]
            for k in range(KC):
                nc.sync.dma_start(out=w_v[k], in_=w_v_d[k])
            b_qk = consts.tile([128, H], F32, tag="bqk")
            nc.sync.dma_start(out=b_qk, in_=b_qk_d[:, :])
            b_v = consts.tile([128, MD], F32, tag="bv")
            nc.sync.dma_start(out=b_v, in_=b_v_d[:, :])
            b_out = consts.tile([128, MD], F32, tag="bout")
            nc.sync.dma_start(out=b_out, in_=b_out_d[:, :])
            ident = consts.tile([128, 128], F32R, tag="id")
            nc.sync.dma_start(out=ident, in_=ident_d[:, :])

            NSL = [(0, 512), (512, MD - 512)]  # N slices for MD-wide matmuls

            # ---- pair-0 QK projection (hoisted: starts as soon as slab0 + xT land) ----
            qt0 = qkp.tile([128, S], F32R, tag="qt", name="qt0")
            kt0 = qkp.tile([128, S], F32R, tag="kt", name="kt0")
            for dst, slab, bi in ((qt0, slabq0, 0), (kt0, slabk0, NPAIR)):
                qps = ps_big.tile([128, 1024], F32, tag="big", name="qkps0")
                for k in range(KC):
                    for nt in range(2):
                        nc.tensor.matmul(
                            qps[:, 512 * nt:512 * (nt + 1)],
                            slab[:, 128 * k:128 * (k + 1)],
                            xT[k][:, 512 * nt:512 * (nt + 1)],
                            start=(k == 0), stop=(k == KC - 1),
                        )
                nc.scalar.add(dst, qps, b_qk[:, bi:bi + 1])

            # ---- V projection: v[s2, (h d)] = xT.T @ W_v + b_v ----
            v_t = [vp.tile([128, MD], F32R, tag=f"v{mt}", name=f"v{mt}") for mt in range(ST)]
            for mt in range(ST):
                ps = ps_t.tile([128, 1024], F32, tag="pt", name=f"vps{mt}")
                for k in range(KC):
                    for (n0, nn) in NSL:
                        nc.tensor.matmul(
                            ps[:, n0:n0 + nn],
                            xT[k][:, 128 * mt:128 * (mt + 1)],
                            w_v[k][:, n0:n0 + nn],
                            start=(k == 0), stop=(k == KC - 1),
                        )
                nc.vector.tensor_add(v_t[mt], ps[:, :MD], b_v)

            w_out = [consts.tile([128, MD], F32R, tag=f"wo{k}", name=f"wo{k}") for k in range(KC)]
            for k in range(KC):
                nc.sync.dma_start(out=w_out[k], in_=w_out_d[k])

            # wT chunk layout: chunk j occupies cols [1024j, 1024(j+1))
            wT = wtp.tile([128, ST * 1024], F32R, tag="wT")
            wT_v = wT.rearrange("p (j q) -> p j q", j=ST)

            # ---- per head-pair ----
            at_all = []
            for c in range(NPAIR):
                if c == 0:
                    slabq, slabk = None, None
                else:
                    slabq = slabp.tile([128, MD], F32R, tag="slabq")
                    nc.sync.dma_start(out=slabq, in_=w_qk_d[c].rearrange("p k c -> p (k c)"))
                    slabk = slabp.tile([128, MD], F32R, tag="slabk")
                    nc.sync.dma_start(out=slabk, in_=w_qk_d[NPAIR + c].rearrange("p k c -> p (k c)"))

                if c == 0:
                    qt, kt = qt0, kt0
                else:
                    qt = qkp.tile([128, S], F32R, tag="qt")
                    kt = qkp.tile([128, S], F32R, tag="kt")
                    for dst, slab, bi in ((qt, slabq, c), (kt, slabk, NPAIR + c)):
                        ps = ps_big.tile([128, 1024], F32, tag="big", name="qkps")
                        for k in range(KC):
                            for nt in range(2):
                                nc.tensor.matmul(
                                    ps[:, 512 * nt:512 * (nt + 1)],
                                    slab[:, 128 * k:128 * (k + 1)],
                                    xT[k][:, 512 * nt:512 * (nt + 1)],
                                    start=(k == 0), stop=(k == KC - 1),
                                )
                        nc.scalar.add(dst, ps, b_qk[:, bi:bi + 1])

                at = atp.tile([128, S], F32R, tag=f"at{c}", name=f"at{c}")
                for r in range(2):
                    h = 2 * c + r
                    for s1t in range(ST):
                        sc = ps_big.tile([128, 1024], F32, tag="big", name="scps")
                        for nt in range(2):
                            nc.tensor.matmul(
                                sc[:, 512 * nt:512 * (nt + 1)],
                                qt[64 * r:64 * (r + 1), 128 * s1t:128 * (s1t + 1)],
                                kt[64 * r:64 * (r + 1), 512 * nt:512 * (nt + 1)],
                                start=True, stop=True,
                            )
                        e = work.tile([128, 1024], F32, tag="e", bufs=3)
                        ssum = scal.tile([128, 1], F32, tag="ss")
                        nc.scalar.activation(
                            e, sc, mybir.ActivationFunctionType.Exp,
                            bias=0.0, scale=SCALE, accum_out=ssum,
                        )
                        rcp = scal.tile([128, 1], F32, tag="rc")
                        nc.vector.reciprocal(rcp, ssum)
                        wtile = work.tile([128, 1024], F32R, tag="w", bufs=3)
                        nc.vector.tensor_scalar_mul(wtile, e, rcp)
                        nc.sync.dma_start(
                            out=wts_d[h, 128 * s1t:128 * (s1t + 1), :], in_=wtile
                        )
                        tp = ps_t.tile([128, 1024], F32R, tag="pt", name="tps")
                        for j in range(ST):
                            nc.tensor.transpose(
                                tp[:, 128 * j:128 * (j + 1)],
                                wtile[:, 128 * j:128 * (j + 1)],
                                ident,
                            )
                        tp_v = tp.rearrange("p (j q) -> p j q", j=ST)
                        JS = 5  # DVE takes chunks 0..4, ACT takes 5..7
                        nc.vector.tensor_copy(
                            wT_v[:, :JS, 128 * s1t:128 * (s1t + 1)],
                            tp_v[:, :JS, :],
                        )
                        nc.scalar.copy(
                            wT_v[:, JS:, 128 * s1t:128 * (s1t + 1)],
                            tp_v[:, JS:, :],
                        )
                    # attention: attnT[d, s1] for head h (fp32r needs dst part 0)
                    pa = ps_t.tile([64, 1024], F32, tag="pt", name=f"pa{h}")
                    for nt in range(2):
                        for j in range(ST):
                            nc.tensor.matmul(
                                pa[:, 512 * nt:512 * (nt + 1)],
                                v_t[j][:, 64 * h:64 * (h + 1)],
                                wT_v[:, j, 512 * nt:512 * (nt + 1)],
                                start=(j == 0), stop=(j == ST - 1),
                            )
                    nc.scalar.copy(at[64 * r:64 * (r + 1), :], pa)
                at_all.append(at)

            # ---- output projection: out[s1, m] = attnT.T @ W_out + b_out ----
            for s1t in range(ST):
                ps = ps_t.tile([128, 1024], F32, tag="pt", name=f"ops{s1t}")
                for cc in range(KC):
                    for (n0, nn) in NSL:
                        nc.tensor.matmul(
                            ps[:, n0:n0 + nn],
                            at_all[cc][:, 128 * s1t:128 * (s1t + 1)],
                            w_out[cc][:, n0:n0 + nn],
                            start=(cc == 0), stop=(cc == KC - 1),
                        )
                osb = work.tile([128, MD], F32, tag="osb")
                nc.vector.tensor_add(osb, ps[:, :MD], b_out)
                nc.sync.dma_start(
                    out=out_d[128 * s1t:128 * (s1t + 1), :], in_=osb
                )

    nc.compile()
    return nc


def prep_inputs(x, W_qkv, b_qkv, W_out, b_out):
    """Host-side resharding/reordering. Returns per-core input maps."""
    x = np.asarray(x, dtype=np.float32)
    W_qkv = np.asarray(W_qkv, dtype=np.float32)
    b_qkv = np.asarray(b_qkv, dtype=np.float32)
    W_out = np.asarray(W_out, dtype=np.float32)
    b_out = np.asarray(b_out, dtype=np.float32)

    # original col index = h*(DK*3) + d*3 + comp
    cols = np.arange(H * DK * 3).reshape(H, DK, 3)
    idx_q = cols[:, :, 0].reshape(-1)     # (h, d) order
    idx_k = cols[:, :, 1].reshape(-1)
    idx_v = cols[:, :, 2].reshape(-1)

    W_QK = np.concatenate([W_qkv[:, idx_q], W_qkv[:, idx_k]], axis=1)  # [768, 1536]
    b_QK = np.concatenate([b_qkv[idx_q], b_qkv[idx_k]])                # [1536]
    W_V = W_qkv[:, idx_v]                                              # [768, 768]
    b_V = b_qkv[idx_v]

    # w_qk tiled: w_qk[m, p, k, cc] = W_QK[128k + p, 128m + cc]
    w_qk = np.ascontiguousarray(
        W_QK.reshape(KC, 128, H, 128).transpose(2, 1, 0, 3)
    )
    b_qk = np.ascontiguousarray(b_QK.reshape(H, 128).T)               # [128, 12]
    w_v = np.ascontiguousarray(W_V.reshape(KC, 128, MD))
    b_v = np.ascontiguousarray(np.broadcast_to(b_V, (128, MD)))
    w_out_t = np.ascontiguousarray(W_out.reshape(KC, 128, MD))
    b_out_b = np.ascontiguousarray(np.broadcast_to(b_out, (128, MD)))
    ident = np.eye(128, dtype=np.float32)

    shared = {
        "w_qk": w_qk, "b_qk": b_qk, "w_v": w_v, "b_v": b_v,
        "w_out": w_out_t, "b_out": b_out_b, "ident": ident,
    }
    in_maps = []
    for b in range(B):
        m = dict(shared)
        m["xT"] = np.ascontiguousarray(x[b].T)
        in_maps.append(m)
    return in_maps


def _get_nc():
    if "nc" not in _BUILD_CACHE:
        _BUILD_CACHE["nc"] = build_nc()
    return _BUILD_CACHE["nc"]


def run(inputs, trace=False, **kw):
    nc = _get_nc()
    in_maps = prep_inputs(**inputs)
    res = run_bass_kernel_spmd(nc, in_maps, core_ids=list(range(B)), trace=trace, **kw)
    out = np.stack([res.results[b]["out"] for b in range(B)])
    weights = np.stack([res.results[b]["weights"] for b in range(B)])
    return (out, weights), res


def kernel(x, W_qkv, b_qkv, W_out, b_out):
    (out, weights), _ = run(
        dict(x=x, W_qkv=W_qkv, b_qkv=b_qkv, W_out=W_out, b_out=b_out)
    )
    return out, weights


# revision 23
# speedup vs baseline: 1.2665x; 1.0245x over previous
"""Bass/Trainium2 kernel for nn_MultiHeadAttention (B=8, S=1024, H=12, D=64, model=768).

Sharding: pure data parallel over batch — core b computes batch element b.
Returns (out, weights) matching the reference.

Matmuls run in fp32r (full PE rate, ~1.6e-4 matmul relerr vs fp32's 4x slowdown).
Softmax (exp, row-sum, normalization) is exact fp32.
"""
import numpy as np
from contextlib import ExitStack

import concourse.bass as bass
import concourse.tile as tile
from concourse import bacc, mybir
from concourse.bass_utils import run_bass_kernel_spmd

F32 = mybir.dt.float32
F32R = mybir.dt.float32r

# Problem constants (hardcoded per contract)
B = 8
S = 1024
MD = 768          # model dim
H = 12            # heads
DK = 64           # head dim
KC = MD // 128    # 6 contraction chunks of 128
ST = S // 128     # 8 s1 tiles
NPAIR = H // 2    # 6 head pairs
SCALE = DK ** -0.5

_BUILD_CACHE = {}


def build_nc():
    """Build the SPMD per-core kernel. Same program on all 8 cores."""
    nc = bacc.Bacc("TRN2", target_bir_lowering=False, debug=False, num_devices=B)

    # ---- DRAM I/O ----  (matmul operands declared fp32r; same bytes as fp32)
    xT_d = nc.dram_tensor("xT", [MD, S], F32R, kind="ExternalInput")
    w_qk_d = nc.dram_tensor("w_qk", [H, 128, KC, 128], F32R, kind="ExternalInput")
    b_qk_d = nc.dram_tensor("b_qk", [128, H], F32, kind="ExternalInput")
    w_v_d = nc.dram_tensor("w_v", [KC, 128, MD], F32R, kind="ExternalInput")
    b_v_d = nc.dram_tensor("b_v", [128, MD], F32, kind="ExternalInput")
    w_out_d = nc.dram_tensor("w_out", [KC, 128, MD], F32R, kind="ExternalInput")
    b_out_d = nc.dram_tensor("b_out", [128, MD], F32, kind="ExternalInput")
    ident_d = nc.dram_tensor("ident", [128, 128], F32R, kind="ExternalInput")

    out_d = nc.dram_tensor("out", [S, MD], F32, kind="ExternalOutput")
    wts_d = nc.dram_tensor("weights", [H, S, S], F32R, kind="ExternalOutput")

    with tile.TileContext(nc) as tc:
        with ExitStack() as ctx:
            consts = ctx.enter_context(tc.tile_pool(name="consts", bufs=1))
            qkp = ctx.enter_context(tc.tile_pool(name="qkp", bufs=2))
            slabp = ctx.enter_context(tc.tile_pool(name="slabp", bufs=2))
            vp = ctx.enter_context(tc.tile_pool(name="vp", bufs=1))
            wtp = ctx.enter_context(tc.tile_pool(name="wtp", bufs=1))
            work = ctx.enter_context(tc.tile_pool(name="work", bufs=2))
            scal = ctx.enter_context(tc.tile_pool(name="scal", bufs=4))
            atp = ctx.enter_context(tc.tile_pool(name="atp", bufs=1))

            ps_big = ctx.enter_context(tc.tile_pool(name="ps_big", bufs=2, space="PSUM"))
            ps_t = ctx.enter_context(tc.tile_pool(name="ps_t", bufs=2, space="PSUM"))

            # ---- pair-0 slabs first: unblocks the first QK projection ASAP ----
            slabq0 = slabp.tile([128, MD], F32R, tag="slabq", name="slabq0")
            nc.sync.dma_start(out=slabq0, in_=w_qk_d[0].rearrange("p k c -> p (k c)"))
            slabk0 = slabp.tile([128, MD], F32R, tag="slabk", name="slabk0")
            nc.sync.dma_start(out=slabk0, in_=w_qk_d[NPAIR].rearrange("p k c -> p (k c)"))

            # ---- constant loads ----
            xT = [consts.tile([128, S], F32R, tag=f"xT{k}", name=f"xT{k}") for k in range(KC)]
            for k in range(KC):
                nc.sync.dma_start(out=xT[k], in_=xT_d[128 * k:128 * (k + 1), :])
            w_v = [consts.tile([128, MD], F32R, tag=f"wv{k}", name=f"wv{k}") for k in range(KC)]
            for k in range(KC):
                nc.sync.dma_start(out=w_v[k], in_=w_v_d[k])
            b_qk = consts.tile([128, H], F32, tag="bqk")
            nc.sync.dma_start(out=b_qk, in_=b_qk_d[:, :])
            b_v = consts.tile([128, MD], F32, tag="bv")
            nc.sync.dma_start(out=b_v, in_=b_v_d[:, :])
            b_out = consts.tile([128, MD], F32, tag="bout")
            nc.sync.dma_start(out=b_out, in_=b_out_d[:, :])
            ident = consts.tile([128, 128], F32R, tag="id")
            nc.sync.dma_start(out=ident, in_=ident_d[:, :])

            NSL = [(0, 512), (512, MD - 512)]  # N slices for MD-wide matmuls

            # ---- pair-0 QK projection (hoisted: starts as soon as slab0 + xT land) ----
            qt0 = qkp.tile([128, S], F32R, tag="qt", name="qt0")
            kt0 = qkp.tile([128, S], F32R, tag="kt", name="kt0")
            for dst, slab, bi in ((qt0, slabq0, 0), (kt0, slabk0, NPAIR)):
                qps = ps_big.tile([128, 1024], F32, tag="big", name="qkps0")
                for k in range(KC):
                    for nt in range(2):
                        nc.tensor.matmul(
                            qps[:, 512 * nt:512 * (nt + 1)],
                            slab[:, 128 * k:128 * (k + 1)],
                            xT[k][:, 512 * nt:512 * (nt + 1)],
                            start=(k == 0), stop=(k == KC - 1),
                        )
                nc.scalar.add(dst, qps, b_qk[:, bi:bi + 1])

            # ---- V projection: v[s2, (h d)] = xT.T @ W_v + b_v ----
            v_t = [vp.tile([128, MD], F32R, tag=f"v{mt}", name=f"v{mt}") for mt in range(ST)]
            for mt in range(ST):
                ps = ps_t.tile([128, 1024], F32, tag="pt", name=f"vps{mt}")
                for k in range(KC):
                    for (n0, nn) in NSL:
                        nc.tensor.matmul(
                            ps[:, n0:n0 + nn],
                            xT[k][:, 128 * mt:128 * (mt + 1)],
                            w_v[k][:, n0:n0 + nn],
                            start=(k == 0), stop=(k == KC - 1),
                        )
                nc.vector.tensor_add(v_t[mt], ps[:, :MD], b_v)

            w_out = [consts.tile([128, MD], F32R, tag=f"wo{k}", name=f"wo{k}") for k in range(KC)]
            for k in range(KC):
                nc.sync.dma_start(out=w_out[k], in_=w_out_d[k])

            # wT chunk layout: chunk j occupies cols [1024j, 1024(j+1))
            wT = wtp.tile([128, ST * 1024], F32R, tag="wT")
            wT_v = wT.rearrange("p (j q) -> p j q", j=ST)

            # ---- per head-pair ----
            def emit_qkproj(cc):
                """Slab DMA + QK projection for pair cc (emitted mid-previous-pair)."""
                sq = slabp.tile([128, MD], F32R, tag="slabq", name=f"slabq{cc}")
                nc.sync.dma_start(out=sq, in_=w_qk_d[cc].rearrange("p k c -> p (k c)"))
                sk = slabp.tile([128, MD], F32R, tag="slabk", name=f"slabk{cc}")
                nc.sync.dma_start(out=sk, in_=w_qk_d[NPAIR + cc].rearrange("p k c -> p (k c)"))
                nqt = qkp.tile([128, S], F32R, tag="qt", name=f"qt{cc}")
                nkt = qkp.tile([128, S], F32R, tag="kt", name=f"kt{cc}")
                for dst, slab, bi in ((nqt, sq, cc), (nkt, sk, NPAIR + cc)):
                    qps2 = ps_big.tile([128, 1024], F32, tag="big", name=f"qkps{cc}")
                    for k in range(KC):
                        for nt in range(2):
                            nc.tensor.matmul(
                                qps2[:, 512 * nt:512 * (nt + 1)],
                                slab[:, 128 * k:128 * (k + 1)],
                                xT[k][:, 512 * nt:512 * (nt + 1)],
                                start=(k == 0), stop=(k == KC - 1),
                            )
                    nc.scalar.add(dst, qps2, b_qk[:, bi:bi + 1])
                return nqt, nkt

            at_all = []
            qkt = {0: (qt0, kt0)}
            for c in range(NPAIR):
                qt, kt = qkt[c]

                at = atp.tile([128, S], F32R, tag=f"at{c}", name=f"at{c}")
                for r in range(2):
                    h = 2 * c + r
                    for s1t in range(ST):
                        sc = ps_big.tile([128, 1024], F32, tag="big", name="scps")
                        for nt in range(2):
                            nc.tensor.matmul(
                                sc[:, 512 * nt:512 * (nt + 1)],
                                qt[64 * r:64 * (r + 1), 128 * s1t:128 * (s1t + 1)],
                                kt[64 * r:64 * (r + 1), 512 * nt:512 * (nt + 1)],
                                start=True, stop=True,
                            )
                        e = work.tile([128, 1024], F32, tag="e", bufs=3)
                        ssum = scal.tile([128, 1], F32, tag="ss")
                        nc.scalar.activation(
                            e, sc, mybir.ActivationFunctionType.Exp,
                            bias=0.0, scale=SCALE, accum_out=ssum,
                        )
                        rcp = scal.tile([128, 1], F32, tag="rc")
                        nc.vector.reciprocal(rcp, ssum)
                        wtile = work.tile([128, 1024], F32R, tag="w", bufs=3)
                        nc.vector.tensor_scalar_mul(wtile, e, rcp)
                        nc.sync.dma_start(
                            out=wts_d[h, 128 * s1t:128 * (s1t + 1), :], in_=wtile
                        )
                        tp = ps_t.tile([128, 1024], F32R, tag="pt", name="tps")
                        for j in range(ST):
                            nc.tensor.transpose(
                                tp[:, 128 * j:128 * (j + 1)],
                                wtile[:, 128 * j:128 * (j + 1)],
                                ident,
                            )
                        tp_v = tp.rearrange("p (j q) -> p j q", j=ST)
                        JS = 5  # DVE takes chunks 0..4, ACT takes 5..7
                        nc.vector.tensor_copy(
                            wT_v[:, :JS, 128 * s1t:128 * (s1t + 1)],
                            tp_v[:, :JS, :],
                        )
                        nc.scalar.copy(
                            wT_v[:, JS:, 128 * s1t:128 * (s1t + 1)],
                            tp_v[:, JS:, :],
                        )
                    # attention: attnT[d, s1] for head h (fp32r needs dst part 0)
                    pa = ps_t.tile([64, 1024], F32, tag="pt", name=f"pa{h}")
                    for nt in range(2):
                        for j in range(ST):
                            nc.tensor.matmul(
                                pa[:, 512 * nt:512 * (nt + 1)],
                                v_t[j][:, 64 * h:64 * (h + 1)],
                                wT_v[:, j, 512 * nt:512 * (nt + 1)],
                                start=(j == 0), stop=(j == ST - 1),
                            )
                    nc.scalar.copy(at[64 * r:64 * (r + 1), :], pa)
                    if r == 0 and c + 1 < NPAIR:
                        qkt[c + 1] = emit_qkproj(c + 1)
                at_all.append(at)

            # ---- output projection: out[s1, m] = attnT.T @ W_out + b_out ----
            for s1t in range(ST):
                ps = ps_t.tile([128, 1024], F32, tag="pt", name=f"ops{s1t}")
                for cc in range(KC):
                    for (n0, nn) in NSL:
                        nc.tensor.matmul(
                            ps[:, n0:n0 + nn],
                            at_all[cc][:, 128 * s1t:128 * (s1t + 1)],
                            w_out[cc][:, n0:n0 + nn],
                            start=(cc == 0), stop=(cc == KC - 1),
                        )
                osb = work.tile([128, MD], F32, tag="osb", bufs=3)
                nc.vector.tensor_add(osb, ps[:, :MD], b_out)
                nc.sync.dma_start(
                    out=out_d[128 * s1t:128 * (s1t + 1), :], in_=osb
                )

    nc.compile()
    return nc


def prep_inputs(x, W_qkv, b_qkv, W_out, b_out):
    """Host-side resharding/reordering. Returns per-core input maps."""
    x = np.asarray(x, dtype=np.float32)
    W_qkv = np.asarray(W_qkv, dtype=np.float32)
    b_qkv = np.asarray(b_qkv, dtype=np.float32)
    W_out = np.asarray(W_out, dtype=np.float32)
    b_out = np.asarray(b_out, dtype=np.float32)

    # original col index = h*(DK*3) + d*3 + comp
    cols = np.arange(H * DK * 3).reshape(H, DK, 3)
    idx_q = cols[:, :, 0].reshape(-1)     # (h, d) order
    idx_k = cols[:, :, 1].reshape(-1)
    idx_v = cols[:, :, 2].reshape(-1)

    W_QK = np.concatenate([W_qkv[:, idx_q], W_qkv[:, idx_k]], axis=1)  # [768, 1536]
    b_QK = np.concatenate([b_qkv[idx_q], b_qkv[idx_k]])                # [1536]
    W_V = W_qkv[:, idx_v]                                              # [768, 768]
    b_V = b_qkv[idx_v]

    # w_qk tiled: w_qk[m, p, k, cc] = W_QK[128k + p, 128m + cc]
    w_qk = np.ascontiguousarray(
        W_QK.reshape(KC, 128, H, 128).transpose(2, 1, 0, 3)
    )
    b_qk = np.ascontiguousarray(b_QK.reshape(H, 128).T)               # [128, 12]
    w_v = np.ascontiguousarray(W_V.reshape(KC, 128, MD))
    b_v = np.ascontiguousarray(np.broadcast_to(b_V, (128, MD)))
    w_out_t = np.ascontiguousarray(W_out.reshape(KC, 128, MD))
    b_out_b = np.ascontiguousarray(np.broadcast_to(b_out, (128, MD)))
    ident = np.eye(128, dtype=np.float32)

    shared = {
        "w_qk": w_qk, "b_qk": b_qk, "w_v": w_v, "b_v": b_v,
        "w_out": w_out_t, "b_out": b_out_b, "ident": ident,
    }
    in_maps = []
    for b in range(B):
        m = dict(shared)
        m["xT"] = np.ascontiguousarray(x[b].T)
        in_maps.append(m)
    return in_maps


def _get_nc():
    if "nc" not in _BUILD_CACHE:
        _BUILD_CACHE["nc"] = build_nc()
    return _BUILD_CACHE["nc"]


def run(inputs, trace=False, **kw):
    nc = _get_nc()
    in_maps = prep_inputs(**inputs)
    res = run_bass_kernel_spmd(nc, in_maps, core_ids=list(range(B)), trace=trace, **kw)
    out = np.stack([res.results[b]["out"] for b in range(B)])
    weights = np.stack([res.results[b]["weights"] for b in range(B)])
    return (out, weights), res


def kernel(x, W_qkv, b_qkv, W_out, b_out):
    (out, weights), _ = run(
        dict(x=x, W_qkv=W_qkv, b_qkv=b_qkv, W_out=W_out, b_out=b_out)
    )
    return out, weights
